# revision 6
# baseline (speedup 1.0000x reference)
"""LinearCondensed kernel for Trainium2 (8 NeuronCores).

Reference computation:
    out[b, o] = sum_f input[b, indx_seqs[o, f]] * weight[o, f] + bias[o]

Strategy: recast the gather-modulated contraction as a dense matmul with a
host-scattered weight matrix (W_dense[o, j] = sum of weight[o, f] with
indx[o, f] == j; out = input @ W_dense^T + bias). Out-features sharded
across the 8 cores (512 outputs/core, input replicated); fp16 operands,
fp32 PSUM accumulation; bias added on host, fp16 output cast back to f32
on host.

Schedule (tuned against the TimelineSim cost model):
  - combined x|w slabs: DRAM tensor XW with row j = [xT[j] | wT[j]] (fp16,
    1024 wide). Uniform 256 KB single-chunk transfers stay transfer-bound
    (360 GB/s) and run ahead of the PE matmul stream with no steady-state
    gaps.
  - a raw pre-context Pool memset feeds PE warmup matmuls from ~750 ns so
    the PE p-state ramp and the first-slab DMA latency overlap.
  - six accumulation chains (b0/b1/b2 full-width 512; block3 split into
    256/192/64-column slices) consume chunks 0..31 in arrival order.
    `tile_wait_until` end-packing holds the block3 chains' late chunks so
    blocks 0-2 stop right after the last slab lands: their PSUM->SBUF
    copies (DVE/ACT) and HWDGE out-DMAs complete while the PE still
    streams block3.
  - block3's three slices go out through SWDGE scatter-adds whose
    descriptors are prepped pre-context on the idle Pool engine (after a
    Pool-side zero-fill of the target rows, sem-ordered) and fired by
    in-tile trigger_dma calls that depend only on each slice's copy.
    The critical tail after the last matmul is just: 64-col copy ->
    trigger -> 16 KB scatter -> DMA-completion sem.
"""

import os
import numpy as np

BATCH = 512
IN_WIDTH = 4096
OUT_FEATURES = 4096
FAN_IN = 128
N_CORES = 8
O_PER_CORE = OUT_FEATURES // N_CORES  # 512
N_JCHUNK = 32

WARMUP = int(os.environ.get("LC_WARMUP", "12"))
TRIGGER_TAIL = os.environ.get("LC_TRIGGER_TAIL", "1") == "1"
TRIGGER_IN_TILE = os.environ.get("LC_TRIGGER_IN_TILE", "1") == "1"
# end-packing: hold chain 3a/3b's chunks >= H_* until their packed slot so
# blocks 0-2 finish right after the last slab lands (their regular out-DMAs
# then complete before the PE stream ends). s_* = target stop times in us.
S_3A = float(os.environ.get("LC_S_3A", "31.0"))
S_3B = float(os.environ.get("LC_S_3B", "32.3"))
S_3C = float(os.environ.get("LC_S_3C", "33.5"))
H_3A = int(os.environ.get("LC_H_3A", "16"))
H_3B = int(os.environ.get("LC_H_3B", "12"))
H_3C = int(os.environ.get("LC_H_3C", "10"))

_NC = {}


def _build(warmup=WARMUP, trigger_tail=TRIGGER_TAIL, in_tile=TRIGGER_IN_TILE,
           s_3a=S_3A, s_3b=S_3B, s_3c=S_3C, h_3a=H_3A, h_3b=H_3B, h_3c=H_3C):
    import concourse.bass as bass
    import concourse.tile as tile
    from concourse import bacc, library_config, mybir

    f32 = mybir.dt.float32
    f16 = mybir.dt.float16
    i16 = mybir.dt.int16

    nc = bacc.Bacc("TRN2", target_bir_lowering=False, debug=False)
    xw = nc.dram_tensor("xw", (IN_WIDTH, 1024), f16, kind="ExternalInput").ap()
    if trigger_tail:
        sc_idx = nc.dram_tensor("sc_idx", (128, 8), i16, kind="ExternalInput").ap()
    out = nc.dram_tensor("out", (BATCH, O_PER_CORE), f16, kind="ExternalOutput").ap()

    def raw_sbuf(name, shape, dtype):
        return nc.alloc_sbuf_tensor(
            name, shape, dtype, target_bir_lowering=nc.target_bir_lowering,
            psum_bank_size_bytes=nc.PSUM_BANK_SIZE_BYTES,
        ).ap()

    # raw pre-context warmup-tile memset: Pool runs it at t~60ns, letting
    # PE warmup matmuls start right after the preamble (~750ns).
    wu_raw = raw_sbuf("wu_raw", [128, 256], f16)
    nc.gpsimd.memset(wu_raw[:], 0.0)

    if trigger_tail:
        idx_t = raw_sbuf("idx_t", [128, 8], i16)
        ot3a = raw_sbuf("ot3a", [128, 1, 256], f16)
        ot3b = raw_sbuf("ot3b", [128, 1, 192], f16)
        ot3c = raw_sbuf("ot3c", [128, 1, 64], f16)
        zt_raw = raw_sbuf("zt_raw", [128, O_PER_CORE], f16)
        idx_sem = nc.alloc_semaphore("idx_dma")
        zt_sem = nc.alloc_semaphore("zt_dma")
        sc_sem = nc.alloc_semaphore("sc_dma")
        nc.gpsimd.load_library(library_config.mlp)
        nc.gpsimd.memset(zt_raw[:], 0.0)
        # idx + zero-fill via Pool SWDGE (HWDGE stays clear for the slab
        # stream); scatter preps after idx lands; zt completion gates the
        # in-tile triggers via Pool FIFO order.
        nc.gpsimd.dma_start(idx_t[:], sc_idx[:]).then_inc(idx_sem, 16)
        nc.gpsimd.dma_start(out[384:512, :], zt_raw[:]).then_inc(zt_sem, 16)
        nc.gpsimd.wait_ge(idx_sem, 16)
        nc.gpsimd.dma_scatter_add(
            out[:, 0:256], ot3a[:], idx_t[:],
            num_idxs=128, num_idxs_reg=128, elem_size=256, elem_step=512,
            prepare_only=True, sem=sc_sem,
        )
        nc.gpsimd.dma_scatter_add(
            out[:, 256:448], ot3b[:], idx_t[:],
            num_idxs=128, num_idxs_reg=128, elem_size=192, elem_step=512,
            prepare_only=True, sem=sc_sem,
        )
        nc.gpsimd.dma_scatter_add(
            out[:, 448:512], ot3c[:], idx_t[:],
            num_idxs=128, num_idxs_reg=128, elem_size=64, elem_step=512,
            prepare_only=True, sem=sc_sem,
        )
        nc.gpsimd.wait_ge(zt_sem, 16)

    with tile.TileContext(nc) as tc:
        with (
            tc.tile_pool(name="xp", bufs=1) as xp,
            tc.tile_pool(name="op", bufs=1) as op,
            tc.tile_pool(name="ps", bufs=1, space=bass.MemorySpace.PSUM) as psp,
        ):
            # ---- input DMA stream (SP engine, all singles) -------------
            slabs = []
            for c in range(N_JCHUNK):
                st = xp.tile([128, 1024], f16, tag=f"s{c}", name=f"s{c}")
                nc.sync.dma_start(st[:], xw[c * 128 : (c + 1) * 128, :])
                slabs.append(st)

            def w_ap(c, lo=0, hi=512):
                return slabs[c][:, 512 + lo : 512 + hi]

            def x_ap(c, bb):
                return slabs[c][:, bass.ts(bb, 128)]

            # ---- PE warmup + zero tile ---------------------------------
            pwu = psp.tile([128, 256], f32, tag="pswu", name="pswu")
            for _ in range(warmup):
                nc.tensor.matmul(
                    pwu[:], wu_raw[:, 0:128], wu_raw[:], start=True, stop=True
                )
            # small filler warmups keep the PE busy until the first slab's
            # data-ready instant so the p-state ramp never resets
            for _ in range(3):
                nc.tensor.matmul(
                    pwu[:, 0:128], wu_raw[:, 0:128], wu_raw[:, 0:128],
                    start=True, stop=True,
                )

            psum = [
                psp.tile([128, O_PER_CORE], f32, tag=f"ps{bb}", name=f"ps{bb}")
                for bb in range(3)
            ]
            ps3a = psp.tile([128, 256], f32, tag="ps3a", name="ps3a")
            ps3b = psp.tile([128, 192], f32, tag="ps3b", name="ps3b")
            ps3c = psp.tile([128, 64], f32, tag="ps3c", name="ps3c")

            # chain spec: (key, psum_ap_fn, mm_emit_fn, dur_us, stop_target)
            def emit_b(bb, c, stop):
                nc.tensor.matmul(psum[bb][:], x_ap(c, bb), w_ap(c),
                                 start=(c == 0), stop=stop)

            def emit_3a(c, stop):
                nc.tensor.matmul(ps3a[:], x_ap(c, 3), w_ap(c, 0, 256),
                                 start=(c == 0), stop=stop)

            def emit_3b(c, stop):
                nc.tensor.matmul(ps3b[:], x_ap(c, 3), w_ap(c, 256, 448),
                                 start=(c == 0), stop=stop)

            def emit_3c(c, stop):
                nc.tensor.matmul(ps3c[:], x_ap(c, 3), w_ap(c, 448, 512),
                                 start=(c == 0), stop=stop)

            # (emit_fn, dur_us, hold_from_chunk, stop_target_us)
            chains = [
                (lambda c, s: emit_b(0, c, s), 0.213, None, 0.0),
                (lambda c, s: emit_b(1, c, s), 0.213, None, 0.0),
                (lambda c, s: emit_b(2, c, s), 0.213, None, 0.0),
                (emit_3a, 0.107, h_3a, s_3a),
                (emit_3b, 0.080, h_3b, s_3b),
                (emit_3c, 0.027, h_3c, s_3c),
            ]

            # ---- main MM stream (chunk-major emission) -----------------
            for c in range(N_JCHUNK):
                for emit, dur, hold_from, s_tgt in chains:
                    w_ms = 0.0
                    if hold_from is not None and c >= hold_from:
                        w_ms = max(0.0, (s_tgt - (N_JCHUNK - 1 - c) * dur) / 1000.0)
                    with tc.tile_wait_until(w_ms, enable=w_ms > 0.0):
                        emit(c, c == N_JCHUNK - 1)

            # ---- copies + out DMAs -------------------------------------
            ot = [
                op.tile([128, O_PER_CORE], f16, tag=f"ot{bb}", name=f"ot{bb}")
                for bb in range(3)
            ]
            nc.vector.tensor_copy(ot[0][:], psum[0][:])
            nc.sync.dma_start(out[0:128, :], ot[0][:])
            nc.scalar.copy(ot[1][:], psum[1][:])
            nc.sync.dma_start(out[128:256, :], ot[1][:])
            nc.vector.tensor_copy(ot[2][:], psum[2][:])
            nc.sync.dma_start(out[256:384, :], ot[2][:])

            if trigger_tail and in_tile:
                # in-tile triggers: clear the prep-tracking list (the preps
                # are raw pre-context instructions invisible to Tile; their
                # FIFO ordering vs the triggers is guaranteed by the Pool
                # queue). signals_writable gives each trigger a WAW dep on
                # its copy, so it fires right after the copy completes —
                # no epilogue-barrier wait on the critical tail; the earlier
                # scatters (+ their DMA-completion sems) hide mid-stream.
                nc.gpsimd._pending_untriggered_insts[0] = []
                nc.vector.tensor_copy(ot3a[:, 0, :], ps3a[:])
                nc.gpsimd.trigger_dma(count=1, signals_writable=[ot3a[:]])
                nc.vector.tensor_copy(ot3b[:, 0, :], ps3b[:])
                nc.gpsimd.trigger_dma(count=1, signals_writable=[ot3b[:]])
                nc.vector.tensor_copy(ot3c[:, 0, :], ps3c[:])
                nc.gpsimd.trigger_dma(count=1, signals_writable=[ot3c[:]])
            elif trigger_tail:
                nc.vector.tensor_copy(ot3a[:, 0, :], ps3a[:])
                nc.vector.tensor_copy(ot3b[:, 0, :], ps3b[:])
                nc.vector.tensor_copy(ot3c[:, 0, :], ps3c[:])
            else:
                ot3a_t = op.tile([128, 256], f16, tag="ot3a", name="ot3a_t")
                nc.vector.tensor_copy(ot3a_t[:], ps3a[:])
                nc.scalar.dma_start(out[384:512, 0:256], ot3a_t[:])
                ot3b_t = op.tile([128, 192], f16, tag="ot3b", name="ot3b_t")
                nc.vector.tensor_copy(ot3b_t[:], ps3b[:])
                nc.scalar.dma_start(out[384:512, 256:448], ot3b_t[:])
                ot3c_t = op.tile([128, 64], f16, tag="ot3c", name="ot3c_t")
                nc.vector.tensor_copy(ot3c_t[:], ps3c[:])
                nc.scalar.dma_start(out[384:512, 448:512], ot3c_t[:])

    if trigger_tail:
        if not in_tile:
            nc.gpsimd.trigger_dma(count=None)
        nc.gpsimd.wait_ge(sc_sem, 48)

    nc.compile()
    return nc


def _get_nc(**kw):
    key = tuple(sorted(kw.items()))
    if key not in _NC:
        _NC[key] = _build(**kw)
    return _NC[key]


def _scatter_dense(inputs):
    w = np.asarray(inputs["weight"], dtype=np.float32)
    idx = np.asarray(inputs["indx_seqs"])
    wTd = np.zeros((IN_WIDTH, OUT_FEATURES), np.float32)
    o_idx = np.repeat(np.arange(OUT_FEATURES, dtype=np.intp), FAN_IN)
    np.add.at(wTd, (idx.ravel(), o_idx), w.ravel())
    return wTd


def _prepare_in_maps(inputs, wTd, trigger_tail=TRIGGER_TAIL):
    x = np.asarray(inputs["input"], dtype=np.float32)
    xT16 = np.ascontiguousarray(x.T.astype(np.float16))
    wT16 = wTd.astype(np.float16)
    idxs = np.zeros((16, 8), np.int16)
    for i in range(128):
        idxs[i % 16, i // 16] = 384 + i
    idxs = np.tile(idxs, (8, 1))  # replicated across the 8 Q7 cores

    in_maps = []
    for c in range(N_CORES):
        sl = slice(c * O_PER_CORE, (c + 1) * O_PER_CORE)
        xwc = np.concatenate([xT16, np.ascontiguousarray(wT16[:, sl])], axis=1)
        m = {"xw": np.ascontiguousarray(xwc)}
        if trigger_tail:
            m["sc_idx"] = idxs
        in_maps.append(m)
    return in_maps


def run(inputs, trace=False):
    from concourse.bass_utils import run_bass_kernel_spmd

    wTd = _scatter_dense(inputs)
    in_maps = _prepare_in_maps(inputs, wTd)
    nc = _get_nc()
    # the PJRT execution path fails intermittently; a fresh rebuild + rerun
    # recovers (observed ~once per dozen runs)
    last_exc = None
    for attempt in range(3):
        try:
            res = run_bass_kernel_spmd(
                nc, in_maps, core_ids=list(range(N_CORES)), trace=trace
            )
            break
        except Exception as exc:  # noqa: BLE001 - retry any backend failure
            last_exc = exc
            _NC.clear()
            nc = _get_nc()
    else:
        raise last_exc
    b = np.asarray(inputs["bias"], dtype=np.float32)
    out = np.concatenate(
        [res.results[c]["out"].astype(np.float32) for c in range(N_CORES)], axis=1
    )
    out += b[None, :]
    return out, res


def kernel(**inputs) -> np.ndarray:
    out, _ = run(inputs, trace=False)
    return out
